# revision 2
# baseline (speedup 1.0000x reference)
"""DenseCaptioner LSTM-gate kernel for 8 Trainium2 NeuronCores.

Role-split sharding (halves per-core HBM traffic vs. gate+batch-half
data parallelism):
  cores 0-3  run program VIS: visual + recurrent paths for gate g = core,
             full batch (two 128-row m-tiles)  -> partial logits [256,1024]
  cores 4-7  run program INP: input path for gate g = core-4, full batch
             -> partial logits [256,1024]
Host: logits[g] = vis_part[g] + inp_part[g] + b[g], then sigmoid/tanh gate
math and the prev_c recurrence.

The two programs are dispatched concurrently on disjoint device subsets
through a copy of concourse's PJRT runner that takes an explicit device
list (the stock one hardcodes jax.devices()[:n]).

Layout: batch-major matmuls (activation^T tiles stationary [128,128],
weight k-tiles streaming [128, 512 or 1024]). Hadamard intermediates are
PE-transposed on device (identity shipped from host: gpsimd faults here).
"""

import numpy as np

import jax
from jax.experimental.shard_map import shard_map
from jax.sharding import Mesh, PartitionSpec

import concourse.mybir as mybir
import concourse.tile as tile
from concourse import bacc, bass2jax

B, X, V, MM, VH, H1, H2, G = 256, 12000, 4096, 1024, 1024, 1024, 1024, 4
XP = 12032  # X padded to a multiple of 128 (94 k-tiles)
N_CORES = 8
MT = 2      # m-tiles (batch 256 = 2 x 128)

DT_NAME = "bfloat16"  # matmul dtype: "float32r" or "bfloat16"

_cache = {}


def _mm_dt():
    return getattr(mybir.dt, DT_NAME)


def _np_dt():
    return mybir.dt.np(_mm_dt())


def build_program(role):
    """role "vis": visual+recurrent paths; "inp": input path. Full batch."""
    dt = _mm_dt()
    f32 = mybir.dt.float32
    # fp32r needs rhs free dim >= 256 for full rate; bf16 allows 1024-wide
    n_chunk = 1024 if dt == mybir.dt.bfloat16 else 512

    nc = bacc.Bacc("TRN2", target_bir_lowering=False, debug=False)

    if role == "vis":
        act_specs = {"v1T": V, "v2T": V, "mT": MM, "hT": H2}
        w_specs = {"V1": V, "V2": V, "C1": VH, "C2": MM, "C3": H1,
                   "U1": H2, "U2": MM, "U3": H1}
    else:
        act_specs = {"xT": XP, "mT": MM}
        w_specs = {"W1": XP, "W2": MM, "W3": H1}

    acts_d = {
        name: nc.dram_tensor(name, [128, k // 128 * B], dt, kind="ExternalInput")
        for name, k in act_specs.items()
    }
    wt = {
        name: nc.dram_tensor(name, [k, H1], dt, kind="ExternalInput")
        for name, k in w_specs.items()
    }
    identD = nc.dram_tensor("identD", [128, 128], dt, kind="ExternalInput")
    out = nc.dram_tensor("out", [B, H2], f32, kind="ExternalOutput")

    with tile.TileContext(nc) as tc:
        with (
            tc.tile_pool(name="acts", bufs=1) as acts,
            tc.tile_pool(name="wstream", bufs=6) as wstream,
            tc.tile_pool(name="inter", bufs=1) as inter,
            tc.tile_pool(name="ps", bufs=2, space="PSUM") as ps,
        ):
            # --- resident activations, [128, ktile, mtile, batch] image ---
            def load_act(name):
                dram = acts_d[name]
                ktiles = act_specs[name] // 128
                t = acts.tile([128, ktiles * B], dt, tag=name)
                nc.sync.dma_start(t[:], dram.ap())
                return t.rearrange("p (t m b) -> p t m b", m=MT, b=128)

            act_sb = {name: load_act(name) for name in act_specs}

            ident_dt = acts.tile([128, 128], dt, tag="ident_dt")
            nc.sync.dma_start(ident_dt[:], identD.ap())

            def stream_mm(psums, act, wname):
                """psums[m][128, 1024] = act_m.T @ W for both m-tiles,
                streaming W k-tiles. act(k, m) -> lhsT [128, 128]."""
                ktiles = w_specs[wname] // 128
                w_dram = wt[wname].ap().rearrange("(t p) n -> t p n", p=128)
                for k in range(ktiles):
                    w = wstream.tile([128, H1], dt, tag="w")
                    nc.sync.dma_start(w[:], w_dram[k])
                    for mi in range(MT):
                        for n in range(0, H1, n_chunk):
                            nc.tensor.matmul(
                                psums[mi][:, n:n + n_chunk],
                                act(k, mi),
                                w[:, n:n + n_chunk],
                                start=(k == 0),
                                stop=(k == ktiles - 1),
                            )

            def hadamard_T(pa, pb):
                """qT[m] = transpose(pa[m] * pb[m]) as SBUF image
                [128, 8, 128] per m-tile; frees pa/pb psum slots."""
                qTs = []
                for mi in range(MT):
                    bounce = inter.tile([128, H1], f32, tag="bounce", bufs=2)
                    nc.vector.tensor_copy(bounce[:], pb[mi][:])
                    q = inter.tile([128, H1], dt, tag="q", bufs=2)
                    nc.vector.tensor_mul(q[:], pa[mi][:], bounce[:])
                    qT = inter.tile([128, (H1 // 128) * 128], dt, tag="qT", bufs=4)
                    qTv = qT.rearrange("p (t b) -> p t b", b=128)
                    for j in range(H1 // 128):
                        ptr = ps.tile([128, 128], dt, tag="s1")
                        nc.tensor.transpose(
                            ptr[:], q[:, j * 128:(j + 1) * 128], ident_dt[:]
                        )
                        nc.vector.tensor_copy(qTv[:, j, :], ptr[:])
                    qTs.append(qTv)
                return qTs

            acc = [inter.tile([128, H2], f32, tag="acc", name=f"acc{i}", bufs=2) for i in range(MT)]

            def level23(qT_in, w_m, w_out, first, lvl2_w=None):
                """acc[m] (+)= ((qT_in[@lvl2_w]) * (m @ w_m)) @ w_out."""
                src = qT_in
                if lvl2_w is not None:
                    pa2 = [ps.tile([128, H1], f32, tag="s1", name=f"pa2_{i}") for i in range(MT)]
                    stream_mm(pa2, lambda k, mi: qT_in[mi][:, k, :], lvl2_w)
                    pb2 = [ps.tile([128, H1], f32, tag="s2", name=f"pb2_{i}") for i in range(MT)]
                    stream_mm(
                        pb2, lambda k, mi: act_sb["mT"][:, k, mi, :], w_m
                    )
                    src = hadamard_T(pa2, pb2)
                l3 = [ps.tile([128, H2], f32, tag="s2", name=f"l3_{i}") for i in range(MT)]
                stream_mm(l3, lambda k, mi: src[mi][:, k, :], w_out)
                for mi in range(MT):
                    if first:
                        nc.vector.tensor_copy(acc[mi][:], l3[mi][:])
                    else:
                        nc.vector.tensor_add(acc[mi][:], acc[mi][:], l3[mi][:])

            def level1(a_name, b_name, w_a, w_b):
                pa = [ps.tile([128, H1], f32, tag="s1", name=f"pa_{i}") for i in range(MT)]
                stream_mm(pa, lambda k, mi: act_sb[a_name][:, k, mi, :], w_a)
                pb = [ps.tile([128, H1], f32, tag="s2", name=f"pb_{i}") for i in range(MT)]
                stream_mm(pb, lambda k, mi: act_sb[b_name][:, k, mi, :], w_b)
                return hadamard_T(pa, pb)

            if role == "vis":
                t1T = level1("v1T", "v2T", "V1", "V2")
                level23(t1T, "C2", "C3", first=True, lvl2_w="C1")
                hqT = level1("hT", "mT", "U1", "U2")
                level23(hqT, None, "U3", first=False)
            else:
                xqT = level1("xT", "mT", "W1", "W2")
                level23(xqT, None, "W3", first=True)

            out_v = out.ap().rearrange("(m p) n -> m p n", p=128)
            for mi in range(MT):
                nc.sync.dma_start(out_v[mi], acc[mi][:])

    nc.compile()
    return nc


def _make_runner(nc, devices):
    """Adapted from concourse.bass2jax.run_bass_via_pjrt: same lowering,
    but runs on an explicit device subset and returns unmaterialized jax
    arrays so two programs can be dispatched concurrently."""
    bass2jax.install_neuronx_cc_hook()

    assert nc.dbg_addr is None
    partition_name = (
        nc.partition_id_tensor.name if nc.partition_id_tensor else None
    )

    in_names, out_names, out_avals, zero_outs = [], [], [], []
    for alloc in nc.m.functions[0].allocations:
        if not isinstance(alloc, mybir.MemoryLocationSet):
            continue
        name = alloc.memorylocations[0].name
        if alloc.kind == "ExternalInput":
            if name != partition_name:
                in_names.append(name)
        elif alloc.kind == "ExternalOutput":
            shape = tuple(alloc.tensor_shape)
            dtype = mybir.dt.np(alloc.dtype)
            out_names.append(name)
            out_avals.append(jax.core.ShapedArray(shape, dtype))
            zero_outs.append(np.zeros(shape, dtype))
    n_params = len(in_names)
    n_outs = len(out_avals)
    in_names.extend(out_names)
    if partition_name is not None:
        in_names.append(partition_name)
    donate = tuple(range(n_params, n_params + n_outs))

    def _body(*args):
        operands = list(args)
        if partition_name is not None:
            operands.append(bass2jax.partition_id_tensor())
        outs = bass2jax._bass_exec_p.bind(
            *operands,
            out_avals=tuple(out_avals),
            in_names=tuple(in_names),
            out_names=tuple(out_names),
            lowering_input_output_aliases=(),
            sim_require_finite=True,
            sim_require_nnan=True,
            nc=nc,
        )
        return tuple(outs)

    n_cores = len(devices)
    mesh = Mesh(np.asarray(devices), ("core",))
    in_specs = (PartitionSpec("core"),) * (n_params + n_outs)
    out_specs = (PartitionSpec("core"),) * n_outs
    sharded = jax.jit(
        shard_map(
            _body, mesh=mesh, in_specs=in_specs, out_specs=out_specs,
            check_rep=False,
        ),
        donate_argnums=donate,
        keep_unused=True,
    )

    def run(in_maps):
        assert len(in_maps) == n_cores
        concat_in = [
            np.concatenate(
                [np.asarray(in_maps[c][name]) for c in range(n_cores)], axis=0
            )
            for name in in_names[:n_params]
        ]
        concat_zeros = [
            np.zeros((n_cores * z.shape[0], *z.shape[1:]), z.dtype)
            for z in zero_outs
        ]
        out_arrs = sharded(*concat_in, *concat_zeros)
        return out_names, out_avals, out_arrs

    return run


def _tile_actT(a, kdim):
    """[256 batch, K<=kdim] -> SBUF image [128, (kdim/128) * 256]:
    (p, (t*2+mi)*128+b) = a[mi*128+b, t*128+p], contiguous per partition."""
    ktiles = kdim // 128
    a = np.asarray(a, np.float32)
    if a.shape[1] < kdim:
        a = np.pad(a, ((0, 0), (0, kdim - a.shape[1])))
    # [2m, 128b, ktiles, 128p] -> [128p, ktiles, 2m, 128b]
    r = a.reshape(MT, 128, ktiles, 128).transpose(3, 2, 0, 1)
    return np.ascontiguousarray(r.reshape(128, ktiles * B), dtype=_np_dt())


def kernel(prev_h, prev_c, x, m, v1, v2, V1, V2, C1, C2, C3, W1, W2, W3, U1, U2, U3, b):
    npdt = _np_dt()
    if "runners" not in _cache:
        devs = jax.devices()
        nc_vis = build_program("vis")
        nc_inp = build_program("inp")
        _cache["runners"] = (
            _make_runner(nc_vis, devs[0:4]),
            _make_runner(nc_inp, devs[4:8]),
        )
        _cache["ncs"] = (nc_vis, nc_inp)
    run_vis, run_inp = _cache["runners"]

    ident = np.eye(128, dtype=np.float32).astype(npdt)

    v1T_img = _tile_actT(v1, V)
    v2T_img = _tile_actT(v2, V)
    mT_img = _tile_actT(m, MM)
    hT_img = _tile_actT(prev_h, H2)
    xT_img = _tile_actT(x, XP)

    vis_maps, inp_maps = [], []
    for g in range(G):
        vis_maps.append({
            "v1T": v1T_img, "v2T": v2T_img, "mT": mT_img, "hT": hT_img,
            "V1": np.ascontiguousarray(V1[g], dtype=npdt),
            "V2": np.ascontiguousarray(V2[g], dtype=npdt),
            "C1": np.ascontiguousarray(C1[g], dtype=npdt),
            "C2": np.ascontiguousarray(C2[g], dtype=npdt),
            "C3": np.ascontiguousarray(C3[g], dtype=npdt),
            "U1": np.ascontiguousarray(U1[g], dtype=npdt),
            "U2": np.ascontiguousarray(U2[g], dtype=npdt),
            "U3": np.ascontiguousarray(U3[g], dtype=npdt),
            "identD": ident,
        })
        w1_pad = np.zeros((XP, H1), np.float32)
        w1_pad[:X] = np.asarray(W1[g], np.float32)
        inp_maps.append({
            "xT": xT_img, "mT": mT_img,
            "W1": np.ascontiguousarray(w1_pad, dtype=npdt),
            "W2": np.ascontiguousarray(W2[g], dtype=npdt),
            "W3": np.ascontiguousarray(W3[g], dtype=npdt),
            "identD": ident,
        })

    _cache["last_in_maps"] = (vis_maps, inp_maps)

    # dispatch both programs; they run concurrently on disjoint cores
    vnames, vavals, vouts = run_vis(vis_maps)
    inames, iavals, iouts = run_inp(inp_maps)

    vis_out = np.asarray(vouts[0]).reshape(G, B, H2)
    inp_out = np.asarray(iouts[0]).reshape(G, B, H2)

    logits = vis_out + inp_out + np.asarray(b, np.float32)[:, None, :]

    def sigmoid(z):
        return 1.0 / (1.0 + np.exp(-z))

    i = sigmoid(logits[0])
    f = sigmoid(logits[1])
    o = sigmoid(logits[2])
    cg = np.tanh(logits[3])
    prev_c = np.asarray(prev_c, np.float32)
    new_c = f * prev_c + i * cg
    new_h = o * np.tanh(prev_c)
    return new_h.astype(np.float32), new_c.astype(np.float32)



# revision 3
# speedup vs baseline: 1.4326x; 1.4326x over previous
"""DenseCaptioner LSTM-gate kernel for 8 Trainium2 NeuronCores.

Role-split sharding (halves per-core HBM traffic vs. gate+batch-half
data parallelism):
  cores 0-3  run program VIS: visual + recurrent paths for gate g = core,
             full batch (two 128-row m-tiles)  -> partial logits [256,1024]
  cores 4-7  run program INP: input path for gate g = core-4, full batch
             -> partial logits [256,1024]
Host: logits[g] = vis_part[g] + inp_part[g] + b[g], then sigmoid/tanh gate
math and the prev_c recurrence.

The two programs are dispatched concurrently on disjoint device subsets
through a copy of concourse's PJRT runner that takes an explicit device
list (the stock one hardcodes jax.devices()[:n]).

Layout: batch-major matmuls (activation^T tiles stationary [128,128],
weight k-tiles streaming [128, 512 or 1024]). Hadamard intermediates are
PE-transposed on device (identity shipped from host: gpsimd faults here).
"""

import numpy as np

import jax
from jax.experimental.shard_map import shard_map
from jax.sharding import Mesh, PartitionSpec

import concourse.mybir as mybir
import concourse.tile as tile
from concourse import bacc, bass2jax

B, X, V, MM, VH, H1, H2, G = 256, 12000, 4096, 1024, 1024, 1024, 1024, 4
XP = 12032  # X padded to a multiple of 128 (94 k-tiles)
N_CORES = 8
MT = 2      # m-tiles (batch 256 = 2 x 128)

DT_NAME = "bfloat16"  # matmul dtype: "float32r" or "bfloat16"

_cache = {}


def _mm_dt():
    return getattr(mybir.dt, DT_NAME)


def _np_dt():
    return mybir.dt.np(_mm_dt())


def build_program(role):
    """role "vis": visual+recurrent paths; "inp": input path. Full batch."""
    dt = _mm_dt()
    f32 = mybir.dt.float32
    # rhs free dim: >=256 for full fp32r rate, <=512 to fit one PSUM bank
    # (matmul output cannot span banks: s3d3_mm_num_elements)
    n_chunk = 512

    nc = bacc.Bacc("TRN2", target_bir_lowering=False, debug=False)

    if role == "vis":
        act_specs = {"v1T": V, "v2T": V, "mT": MM, "hT": H2}
        w_specs = {"V1": V, "V2": V, "C1": VH, "C2": MM, "C3": H1,
                   "U1": H2, "U2": MM, "U3": H1}
    else:
        act_specs = {"xT": XP, "mT": MM}
        w_specs = {"W1": XP, "W2": MM, "W3": H1}

    acts_d = {
        name: nc.dram_tensor(name, [128, k // 128 * B], dt, kind="ExternalInput")
        for name, k in act_specs.items()
    }
    wt = {
        name: nc.dram_tensor(name, [k, H1], dt, kind="ExternalInput")
        for name, k in w_specs.items()
    }
    identD = nc.dram_tensor("identD", [128, 128], dt, kind="ExternalInput")
    out = nc.dram_tensor("out", [B, H2], f32, kind="ExternalOutput")

    with tile.TileContext(nc) as tc:
        with (
            tc.tile_pool(name="acts", bufs=1) as acts,
            tc.tile_pool(name="wstream", bufs=6) as wstream,
            tc.tile_pool(name="inter", bufs=1) as inter,
            tc.tile_pool(name="ps", bufs=2, space="PSUM") as ps,
        ):
            # --- resident activations, [128, ktile, mtile, batch] image ---
            def load_act(name):
                dram = acts_d[name]
                ktiles = act_specs[name] // 128
                t = acts.tile([128, ktiles * B], dt, tag=name)
                nc.sync.dma_start(t[:], dram.ap())
                return t.rearrange("p (t m b) -> p t m b", m=MT, b=128)

            act_sb = {name: load_act(name) for name in act_specs}

            ident_dt = acts.tile([128, 128], dt, tag="ident_dt")
            nc.sync.dma_start(ident_dt[:], identD.ap())

            def stream_mm(psums, act, wname):
                """psums[m][128, 1024] = act_m.T @ W for both m-tiles,
                streaming W k-tiles. act(k, m) -> lhsT [128, 128]."""
                ktiles = w_specs[wname] // 128
                w_dram = wt[wname].ap().rearrange("(t p) n -> t p n", p=128)
                for k in range(ktiles):
                    w = wstream.tile([128, H1], dt, tag="w")
                    nc.sync.dma_start(w[:], w_dram[k])
                    for mi in range(MT):
                        for n in range(0, H1, n_chunk):
                            nc.tensor.matmul(
                                psums[mi][:, n:n + n_chunk],
                                act(k, mi),
                                w[:, n:n + n_chunk],
                                start=(k == 0),
                                stop=(k == ktiles - 1),
                            )

            def hadamard_T(pa, pb):
                """qT[m] = transpose(pa[m] * pb[m]) as SBUF image
                [128, 8, 128] per m-tile; frees pa/pb psum slots."""
                qTs = []
                for mi in range(MT):
                    bounce = inter.tile([128, H1], f32, tag="bounce", bufs=2)
                    nc.vector.tensor_copy(bounce[:], pb[mi][:])
                    q = inter.tile([128, H1], dt, tag="q", bufs=2)
                    nc.vector.tensor_mul(q[:], pa[mi][:], bounce[:])
                    qT = inter.tile([128, (H1 // 128) * 128], dt, tag="qT", bufs=4)
                    qTv = qT.rearrange("p (t b) -> p t b", b=128)
                    for j in range(H1 // 128):
                        ptr = ps.tile([128, 128], dt, tag="s1")
                        nc.tensor.transpose(
                            ptr[:], q[:, j * 128:(j + 1) * 128], ident_dt[:]
                        )
                        nc.vector.tensor_copy(qTv[:, j, :], ptr[:])
                    qTs.append(qTv)
                return qTs

            acc = [inter.tile([128, H2], f32, tag="acc", name=f"acc{i}", bufs=2) for i in range(MT)]

            def level23(qT_in, w_m, w_out, first, lvl2_w=None):
                """acc[m] (+)= ((qT_in[@lvl2_w]) * (m @ w_m)) @ w_out."""
                src = qT_in
                if lvl2_w is not None:
                    pa2 = [ps.tile([128, H1], f32, tag="s1", name=f"pa2_{i}") for i in range(MT)]
                    stream_mm(pa2, lambda k, mi: qT_in[mi][:, k, :], lvl2_w)
                    pb2 = [ps.tile([128, H1], f32, tag="s2", name=f"pb2_{i}") for i in range(MT)]
                    stream_mm(
                        pb2, lambda k, mi: act_sb["mT"][:, k, mi, :], w_m
                    )
                    src = hadamard_T(pa2, pb2)
                l3 = [ps.tile([128, H2], f32, tag="s2", name=f"l3_{i}") for i in range(MT)]
                stream_mm(l3, lambda k, mi: src[mi][:, k, :], w_out)
                for mi in range(MT):
                    if first:
                        nc.vector.tensor_copy(acc[mi][:], l3[mi][:])
                    else:
                        nc.vector.tensor_add(acc[mi][:], acc[mi][:], l3[mi][:])

            def level1(a_name, b_name, w_a, w_b):
                pa = [ps.tile([128, H1], f32, tag="s1", name=f"pa_{i}") for i in range(MT)]
                stream_mm(pa, lambda k, mi: act_sb[a_name][:, k, mi, :], w_a)
                pb = [ps.tile([128, H1], f32, tag="s2", name=f"pb_{i}") for i in range(MT)]
                stream_mm(pb, lambda k, mi: act_sb[b_name][:, k, mi, :], w_b)
                return hadamard_T(pa, pb)

            if role == "vis":
                t1T = level1("v1T", "v2T", "V1", "V2")
                level23(t1T, "C2", "C3", first=True, lvl2_w="C1")
                hqT = level1("hT", "mT", "U1", "U2")
                level23(hqT, None, "U3", first=False)
            else:
                xqT = level1("xT", "mT", "W1", "W2")
                level23(xqT, None, "W3", first=True)

            out_v = out.ap().rearrange("(m p) n -> m p n", p=128)
            for mi in range(MT):
                nc.sync.dma_start(out_v[mi], acc[mi][:])

    nc.compile()
    return nc


def _make_runner(nc, devices):
    """Adapted from concourse.bass2jax.run_bass_via_pjrt: same lowering,
    but runs on an explicit device subset and returns unmaterialized jax
    arrays so two programs can be dispatched concurrently."""
    bass2jax.install_neuronx_cc_hook()

    assert nc.dbg_addr is None
    partition_name = (
        nc.partition_id_tensor.name if nc.partition_id_tensor else None
    )

    in_names, out_names, out_avals, zero_outs = [], [], [], []
    for alloc in nc.m.functions[0].allocations:
        if not isinstance(alloc, mybir.MemoryLocationSet):
            continue
        name = alloc.memorylocations[0].name
        if alloc.kind == "ExternalInput":
            if name != partition_name:
                in_names.append(name)
        elif alloc.kind == "ExternalOutput":
            shape = tuple(alloc.tensor_shape)
            dtype = mybir.dt.np(alloc.dtype)
            out_names.append(name)
            out_avals.append(jax.core.ShapedArray(shape, dtype))
            zero_outs.append(np.zeros(shape, dtype))
    n_params = len(in_names)
    n_outs = len(out_avals)
    in_names.extend(out_names)
    if partition_name is not None:
        in_names.append(partition_name)
    donate = tuple(range(n_params, n_params + n_outs))

    def _body(*args):
        operands = list(args)
        if partition_name is not None:
            operands.append(bass2jax.partition_id_tensor())
        outs = bass2jax._bass_exec_p.bind(
            *operands,
            out_avals=tuple(out_avals),
            in_names=tuple(in_names),
            out_names=tuple(out_names),
            lowering_input_output_aliases=(),
            sim_require_finite=True,
            sim_require_nnan=True,
            nc=nc,
        )
        return tuple(outs)

    n_cores = len(devices)
    mesh = Mesh(np.asarray(devices), ("core",))
    in_specs = (PartitionSpec("core"),) * (n_params + n_outs)
    out_specs = (PartitionSpec("core"),) * n_outs
    sharded = jax.jit(
        shard_map(
            _body, mesh=mesh, in_specs=in_specs, out_specs=out_specs,
            check_rep=False,
        ),
        donate_argnums=donate,
        keep_unused=True,
    )

    def run(in_maps):
        assert len(in_maps) == n_cores
        concat_in = [
            np.concatenate(
                [np.asarray(in_maps[c][name]) for c in range(n_cores)], axis=0
            )
            for name in in_names[:n_params]
        ]
        concat_zeros = [
            np.zeros((n_cores * z.shape[0], *z.shape[1:]), z.dtype)
            for z in zero_outs
        ]
        out_arrs = sharded(*concat_in, *concat_zeros)
        return out_names, out_avals, out_arrs

    return run


def _tile_actT(a, kdim):
    """[256 batch, K<=kdim] -> SBUF image [128, (kdim/128) * 256]:
    (p, (t*2+mi)*128+b) = a[mi*128+b, t*128+p], contiguous per partition."""
    ktiles = kdim // 128
    a = np.asarray(a, np.float32)
    if a.shape[1] < kdim:
        a = np.pad(a, ((0, 0), (0, kdim - a.shape[1])))
    # [2m, 128b, ktiles, 128p] -> [128p, ktiles, 2m, 128b]
    r = a.reshape(MT, 128, ktiles, 128).transpose(3, 2, 0, 1)
    return np.ascontiguousarray(r.reshape(128, ktiles * B), dtype=_np_dt())


def kernel(prev_h, prev_c, x, m, v1, v2, V1, V2, C1, C2, C3, W1, W2, W3, U1, U2, U3, b):
    npdt = _np_dt()
    if "runners" not in _cache:
        devs = jax.devices()
        nc_vis = build_program("vis")
        nc_inp = build_program("inp")
        _cache["runners"] = (
            _make_runner(nc_vis, devs[0:4]),
            _make_runner(nc_inp, devs[4:8]),
        )
        _cache["ncs"] = (nc_vis, nc_inp)
    run_vis, run_inp = _cache["runners"]

    ident = np.eye(128, dtype=np.float32).astype(npdt)

    v1T_img = _tile_actT(v1, V)
    v2T_img = _tile_actT(v2, V)
    mT_img = _tile_actT(m, MM)
    hT_img = _tile_actT(prev_h, H2)
    xT_img = _tile_actT(x, XP)

    vis_maps, inp_maps = [], []
    for g in range(G):
        vis_maps.append({
            "v1T": v1T_img, "v2T": v2T_img, "mT": mT_img, "hT": hT_img,
            "V1": np.ascontiguousarray(V1[g], dtype=npdt),
            "V2": np.ascontiguousarray(V2[g], dtype=npdt),
            "C1": np.ascontiguousarray(C1[g], dtype=npdt),
            "C2": np.ascontiguousarray(C2[g], dtype=npdt),
            "C3": np.ascontiguousarray(C3[g], dtype=npdt),
            "U1": np.ascontiguousarray(U1[g], dtype=npdt),
            "U2": np.ascontiguousarray(U2[g], dtype=npdt),
            "U3": np.ascontiguousarray(U3[g], dtype=npdt),
            "identD": ident,
        })
        w1_pad = np.zeros((XP, H1), np.float32)
        w1_pad[:X] = np.asarray(W1[g], np.float32)
        inp_maps.append({
            "xT": xT_img, "mT": mT_img,
            "W1": np.ascontiguousarray(w1_pad, dtype=npdt),
            "W2": np.ascontiguousarray(W2[g], dtype=npdt),
            "W3": np.ascontiguousarray(W3[g], dtype=npdt),
            "identD": ident,
        })

    _cache["last_in_maps"] = (vis_maps, inp_maps)

    # dispatch both programs; they run concurrently on disjoint cores
    vnames, vavals, vouts = run_vis(vis_maps)
    inames, iavals, iouts = run_inp(inp_maps)

    vis_out = np.asarray(vouts[0]).reshape(G, B, H2)
    inp_out = np.asarray(iouts[0]).reshape(G, B, H2)

    logits = vis_out + inp_out + np.asarray(b, np.float32)[:, None, :]

    def sigmoid(z):
        return 1.0 / (1.0 + np.exp(-z))

    i = sigmoid(logits[0])
    f = sigmoid(logits[1])
    o = sigmoid(logits[2])
    cg = np.tanh(logits[3])
    prev_c = np.asarray(prev_c, np.float32)
    new_c = f * prev_c + i * cg
    new_h = o * np.tanh(prev_c)
    return new_h.astype(np.float32), new_c.astype(np.float32)



# revision 5
# speedup vs baseline: 1.6002x; 1.1170x over previous
"""DenseCaptioner LSTM-gate kernel for 8 Trainium2 NeuronCores.

Role-split sharding (no weight replication: each weight matrix is read
from HBM exactly once across the machine):
  cores 0-3  run program VIS: visual + recurrent paths for gate g = core,
             full batch (two 128-row m-tiles)  -> partial logits [256,1024]
  cores 4-7  run program INP: input path for gate g = core-4, full batch
             -> partial logits [256,1024]
Host: logits[g] = vis_part[g] + inp_part[g] + b[g], then sigmoid/tanh gate
math and the prev_c recurrence.

All matmul operands are bf16 (PSUM accumulation stays fp32): fp32r and
bf16 both stream 1 row/cycle on the TRN2 PE, so bf16's win is purely the
halved HBM traffic, which was the binding roofline (emulated end-to-end
rel err 4.6e-3 vs the 2e-2 gate).

Schedule (per core): independent m-projections (C2/U2 resp. W2) run
first into PSUM and are evacuated to SBUF - they keep the PE busy while
the big activation images stream in (activation DMAs are chunked per
4 k-tiles and issued from the otherwise-idle Activation queue so the
sync queue's ~600ns/DMA issue serialization doesn't gate startup), and
the later hadamards multiply PSUM x SBUF directly with no bounce copy.
The U1 stream is emitted between H1's DVE muls and its PE transposes to
fill that dependency stall. C3+U3 share one open PSUM accumulation
group. PSUM budget: 2 tags x 2 slots x 2 banks = all 8 banks; transpose
scratch reuses freed slots of the opposite tag.

The two programs are dispatched concurrently on disjoint device subsets
through a copy of concourse's PJRT runner that takes an explicit device
list (the stock one hardcodes jax.devices()[:n]).
"""

import numpy as np

import jax
from jax.experimental.shard_map import shard_map
from jax.sharding import Mesh, PartitionSpec

import concourse.mybir as mybir
import concourse.tile as tile
from concourse import bacc, bass2jax

B, X, V, MM, VH, H1, H2, G = 256, 12000, 4096, 1024, 1024, 1024, 1024, 4
XP = 12032  # X padded to a multiple of 128 (94 k-tiles)
N_CORES = 8
MT = 2      # m-tiles (batch 256 = 2 x 128)

DT_NAME = "bfloat16"  # matmul dtype: "float32r" or "bfloat16"

_cache = {}


def _mm_dt():
    return getattr(mybir.dt, DT_NAME)


def _np_dt():
    return mybir.dt.np(_mm_dt())


def build_program(role):
    """role "vis": visual+recurrent paths; "inp": input path. Full batch."""
    dt = _mm_dt()
    f32 = mybir.dt.float32
    # rhs free dim: >=256 for full fp32r rate, <=512 to fit one PSUM bank
    # (matmul output cannot span banks: s3d3_mm_num_elements)
    n_chunk = 512

    nc = bacc.Bacc("TRN2", target_bir_lowering=False, debug=False)

    if role == "vis":
        act_specs = {"mT": MM, "hT": H2, "v1T": V, "v2T": V}
        w_specs = {"V1": V, "V2": V, "C1": VH, "C2": MM, "C3": H1,
                   "U1": H2, "U2": MM, "U3": H1}
    else:
        act_specs = {"mT": MM, "xT": XP}
        w_specs = {"W1": XP, "W2": MM, "W3": H1}

    acts_d = {
        name: nc.dram_tensor(name, [128, k // 128 * B], dt, kind="ExternalInput")
        for name, k in act_specs.items()
    }
    wt = {
        name: nc.dram_tensor(name, [k, H1], dt, kind="ExternalInput")
        for name, k in w_specs.items()
    }
    identD = nc.dram_tensor("identD", [128, 128], dt, kind="ExternalInput")
    out = nc.dram_tensor("out", [B, H2], f32, kind="ExternalOutput")

    with tile.TileContext(nc) as tc:
        with (
            tc.tile_pool(name="acts", bufs=1) as acts,
            tc.tile_pool(name="wstream", bufs=12) as wstream,
            tc.tile_pool(name="inter", bufs=1) as inter,
            tc.tile_pool(name="ps", bufs=2, space="PSUM") as ps,
        ):
            act_sb = {}

            def load_act(name, chunk_kt=4):
                """Chunked resident activation load, [128, ktile, mtile,
                batch] image; issued on the Activation queue."""
                dram = acts_d[name]
                ktiles = act_specs[name] // 128
                t = acts.tile([128, ktiles * B], dt, tag=name, name=name)
                for c0 in range(0, ktiles, chunk_kt):
                    c1 = min(c0 + chunk_kt, ktiles)
                    nc.scalar.dma_start(
                        t[:, c0 * B:c1 * B], dram.ap()[:, c0 * B:c1 * B]
                    )
                act_sb[name] = t.rearrange("p (t m b) -> p t m b", m=MT, b=128)

            def act_view(name):
                return lambda k, mi: act_sb[name][:, k, mi, :]

            def qt_view(qTs):
                return lambda k, mi: qTs[mi][:, k, :]

            def stream_mm(act, wname, ptag, psums=None, start_group=True,
                          stop_group=True):
                """psums[m][128, 1024] (+)= act_m.T @ W, streaming W k-tiles.
                act(k, mi) -> lhsT [128, 128]."""
                ktiles = w_specs[wname] // 128
                w_dram = wt[wname].ap().rearrange("(t p) n -> t p n", p=128)
                if psums is None:
                    psums = [
                        ps.tile([128, H1], f32, tag=ptag, name=f"ps_{wname}{i}")
                        for i in range(MT)
                    ]
                for k in range(ktiles):
                    w = wstream.tile([128, H1], dt, tag="w", name=f"w_{wname}{k}")
                    nc.sync.dma_start(w[:], w_dram[k])
                    for mi in range(MT):
                        for n in range(0, H1, n_chunk):
                            nc.tensor.matmul(
                                psums[mi][:, n:n + n_chunk],
                                act(k, mi),
                                w[:, n:n + n_chunk],
                                start=start_group and (k == 0),
                                stop=stop_group and (k == ktiles - 1),
                            )
                return psums

            def evac_sbuf(psums, name):
                """Copy psum accumulators to resident SBUF f32 tiles."""
                sb = []
                for mi in range(MT):
                    s = inter.tile([128, H1], f32, tag=name,
                                   name=f"{name}{mi}", bufs=2)
                    nc.vector.tensor_copy(s[:], psums[mi][:])
                    sb.append(s)
                return sb

            def had_mul(pa, partner_sb=None, bounce_from=None):
                """q[mi] (bf16 SBUF) = pa[mi] (psum) * partner; partner is
                resident SBUF f32, or a psum bounced through SBUF."""
                qs = []
                for mi in range(MT):
                    if partner_sb is None:
                        bnc = inter.tile([128, H1], f32, tag="bounce",
                                         name=f"bounce{mi}", bufs=2)
                        nc.vector.tensor_copy(bnc[:], bounce_from[mi][:])
                        src = bnc
                    else:
                        src = partner_sb[mi]
                    q = inter.tile([128, H1], dt, tag="q", name=f"q{mi}",
                                   bufs=2)
                    nc.vector.tensor_mul(q[:], pa[mi][:], src[:])
                    qs.append(q)
                return qs

            ident_dt = acts.tile([128, 128], dt, tag="ident_dt")

            def had_transp(qs, ttag):
                """qT[m] = transpose(q[m]) as SBUF image [128, 8, 128] per
                m-tile; PE transposes through freed psum slots of ttag."""
                qTs = []
                for mi in range(MT):
                    qT = inter.tile([128, (H1 // 128) * 128], dt, tag="qT",
                                    name=f"qT{mi}", bufs=4)
                    qTv = qT.rearrange("p (t b) -> p t b", b=128)
                    for j in range(H1 // 128):
                        ptr = ps.tile([128, 128], dt, tag=ttag,
                                      name=f"ptr{mi}_{j}")
                        nc.tensor.transpose(
                            ptr[:], qs[mi][:, j * 128:(j + 1) * 128],
                            ident_dt[:]
                        )
                        nc.vector.tensor_copy(qTv[:, j, :], ptr[:])
                    qTs.append(qTv)
                return qTs

            def finish(l3):
                out_v = out.ap().rearrange("(m p) n -> m p n", p=128)
                for mi in range(MT):
                    acc = inter.tile([128, H2], f32, tag="acc",
                                     name=f"acc{mi}", bufs=2)
                    nc.vector.tensor_copy(acc[:], l3[mi][:])
                    nc.scalar.dma_start(out_v[mi], acc[:])

            if role == "vis":
                load_act("mT")
                pc2 = stream_mm(act_view("mT"), "C2", "A")
                c2sb = evac_sbuf(pc2, "c2sb")
                load_act("hT")
                pu2 = stream_mm(act_view("mT"), "U2", "B")
                u2sb = evac_sbuf(pu2, "u2sb")
                nc.scalar.dma_start(ident_dt[:], identD.ap())
                load_act("v1T")
                load_act("v2T")
                pa = stream_mm(act_view("v1T"), "V1", "A")
                pb = stream_mm(act_view("v2T"), "V2", "B")
                q1 = had_mul(pa, bounce_from=pb)        # frees A and B
                pu = stream_mm(act_view("hT"), "U1", "A")  # fills H1 stall
                t1T = had_transp(q1, "B")
                pa2 = stream_mm(qt_view(t1T), "C1", "B")
                q2 = had_mul(pa2, partner_sb=c2sb)      # frees B
                t2T = had_transp(q2, "B")
                l3 = stream_mm(qt_view(t2T), "C3", "B", start_group=True,
                               stop_group=False)
                q3 = had_mul(pu, partner_sb=u2sb)       # frees A; runs || C3
                t3T = had_transp(q3, "A")
                stream_mm(qt_view(t3T), "U3", None, psums=l3,
                          start_group=False, stop_group=True)
                finish(l3)
            else:
                load_act("mT")
                pw2 = stream_mm(act_view("mT"), "W2", "A")
                w2sb = evac_sbuf(pw2, "w2sb")
                nc.scalar.dma_start(ident_dt[:], identD.ap())
                load_act("xT")
                pa = stream_mm(act_view("xT"), "W1", "B")
                q = had_mul(pa, partner_sb=w2sb)        # frees B
                tT = had_transp(q, "A")
                l3 = stream_mm(qt_view(tT), "W3", "B")
                finish(l3)

    nc.compile()
    return nc


def _make_runner(nc, devices):
    """Adapted from concourse.bass2jax.run_bass_via_pjrt: same lowering,
    but runs on an explicit device subset and returns unmaterialized jax
    arrays so two programs can be dispatched concurrently."""
    bass2jax.install_neuronx_cc_hook()

    assert nc.dbg_addr is None
    partition_name = (
        nc.partition_id_tensor.name if nc.partition_id_tensor else None
    )

    in_names, out_names, out_avals, zero_outs = [], [], [], []
    for alloc in nc.m.functions[0].allocations:
        if not isinstance(alloc, mybir.MemoryLocationSet):
            continue
        name = alloc.memorylocations[0].name
        if alloc.kind == "ExternalInput":
            if name != partition_name:
                in_names.append(name)
        elif alloc.kind == "ExternalOutput":
            shape = tuple(alloc.tensor_shape)
            dtype = mybir.dt.np(alloc.dtype)
            out_names.append(name)
            out_avals.append(jax.core.ShapedArray(shape, dtype))
            zero_outs.append(np.zeros(shape, dtype))
    n_params = len(in_names)
    n_outs = len(out_avals)
    in_names.extend(out_names)
    if partition_name is not None:
        in_names.append(partition_name)
    donate = tuple(range(n_params, n_params + n_outs))

    def _body(*args):
        operands = list(args)
        if partition_name is not None:
            operands.append(bass2jax.partition_id_tensor())
        outs = bass2jax._bass_exec_p.bind(
            *operands,
            out_avals=tuple(out_avals),
            in_names=tuple(in_names),
            out_names=tuple(out_names),
            lowering_input_output_aliases=(),
            sim_require_finite=True,
            sim_require_nnan=True,
            nc=nc,
        )
        return tuple(outs)

    n_cores = len(devices)
    mesh = Mesh(np.asarray(devices), ("core",))
    in_specs = (PartitionSpec("core"),) * (n_params + n_outs)
    out_specs = (PartitionSpec("core"),) * n_outs
    sharded = jax.jit(
        shard_map(
            _body, mesh=mesh, in_specs=in_specs, out_specs=out_specs,
            check_rep=False,
        ),
        donate_argnums=donate,
        keep_unused=True,
    )

    def run(in_maps):
        assert len(in_maps) == n_cores
        concat_in = [
            np.concatenate(
                [np.asarray(in_maps[c][name]) for c in range(n_cores)], axis=0
            )
            for name in in_names[:n_params]
        ]
        concat_zeros = [
            np.zeros((n_cores * z.shape[0], *z.shape[1:]), z.dtype)
            for z in zero_outs
        ]
        out_arrs = sharded(*concat_in, *concat_zeros)
        return out_names, out_avals, out_arrs

    return run


def _tile_actT(a, kdim):
    """[256 batch, K<=kdim] -> SBUF image [128, (kdim/128) * 256]:
    (p, (t*2+mi)*128+b) = a[mi*128+b, t*128+p], contiguous per partition."""
    ktiles = kdim // 128
    a = np.asarray(a, np.float32)
    if a.shape[1] < kdim:
        a = np.pad(a, ((0, 0), (0, kdim - a.shape[1])))
    # [2m, 128b, ktiles, 128p] -> [128p, ktiles, 2m, 128b]
    r = a.reshape(MT, 128, ktiles, 128).transpose(3, 2, 0, 1)
    return np.ascontiguousarray(r.reshape(128, ktiles * B), dtype=_np_dt())


def kernel(prev_h, prev_c, x, m, v1, v2, V1, V2, C1, C2, C3, W1, W2, W3, U1, U2, U3, b):
    npdt = _np_dt()
    if "runners" not in _cache:
        devs = jax.devices()
        nc_vis = build_program("vis")
        nc_inp = build_program("inp")
        _cache["runners"] = (
            _make_runner(nc_vis, devs[0:4]),
            _make_runner(nc_inp, devs[4:8]),
        )
        _cache["ncs"] = (nc_vis, nc_inp)
    run_vis, run_inp = _cache["runners"]

    ident = np.eye(128, dtype=np.float32).astype(npdt)

    v1T_img = _tile_actT(v1, V)
    v2T_img = _tile_actT(v2, V)
    mT_img = _tile_actT(m, MM)
    hT_img = _tile_actT(prev_h, H2)
    xT_img = _tile_actT(x, XP)

    vis_maps, inp_maps = [], []
    for g in range(G):
        vis_maps.append({
            "v1T": v1T_img, "v2T": v2T_img, "mT": mT_img, "hT": hT_img,
            "V1": np.ascontiguousarray(V1[g], dtype=npdt),
            "V2": np.ascontiguousarray(V2[g], dtype=npdt),
            "C1": np.ascontiguousarray(C1[g], dtype=npdt),
            "C2": np.ascontiguousarray(C2[g], dtype=npdt),
            "C3": np.ascontiguousarray(C3[g], dtype=npdt),
            "U1": np.ascontiguousarray(U1[g], dtype=npdt),
            "U2": np.ascontiguousarray(U2[g], dtype=npdt),
            "U3": np.ascontiguousarray(U3[g], dtype=npdt),
            "identD": ident,
        })
        w1_pad = np.zeros((XP, H1), np.float32)
        w1_pad[:X] = np.asarray(W1[g], np.float32)
        inp_maps.append({
            "xT": xT_img, "mT": mT_img,
            "W1": np.ascontiguousarray(w1_pad, dtype=npdt),
            "W2": np.ascontiguousarray(W2[g], dtype=npdt),
            "W3": np.ascontiguousarray(W3[g], dtype=npdt),
            "identD": ident,
        })

    _cache["last_in_maps"] = (vis_maps, inp_maps)

    # dispatch both programs; they run concurrently on disjoint cores
    vnames, vavals, vouts = run_vis(vis_maps)
    inames, iavals, iouts = run_inp(inp_maps)

    vis_out = np.asarray(vouts[0]).reshape(G, B, H2)
    inp_out = np.asarray(iouts[0]).reshape(G, B, H2)

    logits = vis_out + inp_out + np.asarray(b, np.float32)[:, None, :]

    def sigmoid(z):
        return 1.0 / (1.0 + np.exp(-z))

    i = sigmoid(logits[0])
    f = sigmoid(logits[1])
    o = sigmoid(logits[2])
    cg = np.tanh(logits[3])
    prev_c = np.asarray(prev_c, np.float32)
    new_c = f * prev_c + i * cg
    new_h = o * np.tanh(prev_c)
    return new_h.astype(np.float32), new_c.astype(np.float32)


# revision 7
# speedup vs baseline: 1.7416x; 1.0884x over previous
"""DenseCaptioner LSTM-gate kernel for 8 Trainium2 NeuronCores.

Role-split sharding (no weight replication: each weight matrix is read
from HBM exactly once across the machine):
  cores 0-3  run program VIS: visual + recurrent paths for gate g = core,
             full batch  -> partial logits^T [1024,256]
  cores 4-7  run program INP: input path for gate g = core-4, full batch
             -> partial logits^T [1024,256]
Host: logits[g] = vis_part[g] + inp_part[g] + b[g], then sigmoid/tanh gate
math and the prev_c recurrence.

All matmul operands are bf16 (PSUM accumulation stays fp32): fp32r and
bf16 both stream 1 row/cycle on the TRN2 PE, so bf16's win is purely the
halved HBM traffic (emulated end-to-end rel err 4.6e-3 vs the 2e-2 gate).

Weight-stationary layout: every matmul uses a 128-column slice of the
streamed weight k-tile as the stationary lhsT and the [128,256]
activation k-tile image as the moving rhs, producing outputs directly in
[h-part, batch] layout. Hadamard products stay in that layout and feed
the next level as the moving rhs - no PE transposes, no identity, and
chunk-granular (128-row) pipelining across the hadamard boundaries. The
LDWEIGHTS pipe (8 x 128-row loads per k-tile) double-buffers under the
8 x 256-row matmuls.

Schedule (per core): independent m-projections (C2/U2 resp. W2) run
first into PSUM and are evacuated to SBUF - they keep the PE busy while
the big activation images stream in (activation DMAs are chunked per
4 k-tiles and issued from the otherwise-idle Activation queue so the
sync queue's ~600ns/DMA issue serialization doesn't gate startup), and
the later hadamards multiply PSUM x SBUF directly with no bounce copy.
The U1 stream is emitted between H1's DVE muls and dependent work to
fill that stall. C3+U3 share one open PSUM accumulation group. PSUM
budget: 2 tags x 4 slots x 1 bank (two 256-wide h-chunks per bank tile)
= all 8 banks.

The two programs are dispatched concurrently on disjoint device subsets
through a copy of concourse's PJRT runner that takes an explicit device
list (the stock one hardcodes jax.devices()[:n]).
"""

import numpy as np

import jax
from jax.experimental.shard_map import shard_map
from jax.sharding import Mesh, PartitionSpec

import concourse.mybir as mybir
import concourse.tile as tile
from concourse import bacc, bass2jax

B, X, V, MM, VH, H1, H2, G = 256, 12000, 4096, 1024, 1024, 1024, 1024, 4
XP = 12032  # X padded to a multiple of 128 (94 k-tiles)
N_CORES = 8
NJ = H1 // 128  # output h-chunks per stream

DT_NAME = "bfloat16"  # matmul dtype: "float32r" or "bfloat16"

_cache = {}


def _mm_dt():
    return getattr(mybir.dt, DT_NAME)


def _np_dt():
    return mybir.dt.np(_mm_dt())


def build_program(role):
    """role "vis": visual+recurrent paths; "inp": input path. Full batch."""
    dt = _mm_dt()
    f32 = mybir.dt.float32

    nc = bacc.Bacc("TRN2", target_bir_lowering=False, debug=False)

    if role == "vis":
        act_specs = {"mT": MM, "hT": H2, "v1T": V, "v2T": V}
        w_specs = {"V1": V, "V2": V, "C1": VH, "C2": MM, "C3": H1,
                   "U1": H2, "U2": MM, "U3": H1}
    else:
        act_specs = {"mT": MM, "xT": XP}
        w_specs = {"W1": XP, "W2": MM, "W3": H1}

    acts_d = {
        name: nc.dram_tensor(name, [128, k // 128 * B], dt, kind="ExternalInput")
        for name, k in act_specs.items()
    }
    wt = {
        name: nc.dram_tensor(name, [k, H1], dt, kind="ExternalInput")
        for name, k in w_specs.items()
    }
    out = nc.dram_tensor("out", [H1, B], f32, kind="ExternalOutput")

    with tile.TileContext(nc) as tc:
        with (
            tc.tile_pool(name="acts", bufs=1) as acts,
            tc.tile_pool(name="wstream", bufs=12) as wstream,
            tc.tile_pool(name="inter", bufs=1) as inter,
            tc.tile_pool(name="ps", bufs=4, space="PSUM") as ps,
        ):
            act_sb = {}

            def load_act(name, chunk_kt=4):
                """Chunked resident activation load, [128, ktile, batch]
                image; issued on the Activation queue."""
                dram = acts_d[name]
                ktiles = act_specs[name] // 128
                t = acts.tile([128, ktiles * B], dt, tag=name, name=name)
                for c0 in range(0, ktiles, chunk_kt):
                    c1 = min(c0 + chunk_kt, ktiles)
                    nc.scalar.dma_start(
                        t[:, c0 * B:c1 * B], dram.ap()[:, c0 * B:c1 * B]
                    )
                act_sb[name] = t.rearrange("p (t x) -> p t x", x=B)

            def act_view(name):
                return lambda k: act_sb[name][:, k, :]

            def q_view(q):
                return lambda k: q[:, k * B:(k + 1) * B]

            # psum: [128, 512] f32 bank tiles, two 256-wide h-chunks each
            def pslice(psums, j):
                return psums[j // 2][:, (j % 2) * B:(j % 2 + 1) * B]

            def stream_mm(rhs, wname, ptag, psums=None, start_group=True,
                          stop_group=True):
                """pslice(psums, j) [128, 256] (+)= W_ktile[:, j*128:...]^T
                @ rhs(k), streaming W k-tiles: weight columns stationary,
                activation image moving."""
                ktiles = w_specs[wname] // 128
                w_dram = wt[wname].ap().rearrange("(t p) n -> t p n", p=128)
                if psums is None:
                    psums = [
                        ps.tile([128, 2 * B], f32, tag=ptag,
                                name=f"ps_{wname}{i}")
                        for i in range(NJ // 2)
                    ]
                for k in range(ktiles):
                    w = wstream.tile([128, H1], dt, tag="w", name=f"w_{wname}{k}")
                    nc.sync.dma_start(w[:], w_dram[k])
                    r = rhs(k)
                    for j in range(NJ):
                        nc.tensor.matmul(
                            pslice(psums, j),
                            w[:, j * 128:(j + 1) * 128],
                            r,
                            # start zeroes the whole 2KB PSUM bank, so only
                            # the first chunk of each bank pair may set it
                            start=start_group and (k == 0) and (j % 2 == 0),
                            stop=stop_group and (k == ktiles - 1),
                        )
                return psums

            def evac_sbuf(psums, name):
                """Copy psum accumulators to a resident SBUF f32 image."""
                s = inter.tile([128, NJ * B], f32, tag=name, name=name)
                for j in range(NJ):
                    nc.vector.tensor_copy(s[:, j * B:(j + 1) * B],
                                          pslice(psums, j))
                return s

            def had_mul(pa, partner_sb=None, bounce_from=None, qname="q"):
                """q [128, NJ*256] bf16 = pa * partner, chunk-granular.
                partner: resident SBUF f32 image, or psum bounced via SBUF."""
                q = inter.tile([128, NJ * B], dt, tag="q", name=qname, bufs=2)
                bnc = None
                if partner_sb is None:
                    bnc = inter.tile([128, NJ * B], f32, tag="bounce",
                                     name=f"bounce_{qname}", bufs=2)
                for j in range(NJ):
                    sl = slice(j * B, (j + 1) * B)
                    if partner_sb is None:
                        nc.vector.tensor_copy(bnc[:, sl],
                                              pslice(bounce_from, j))
                        src = bnc[:, sl]
                    else:
                        src = partner_sb[:, sl]
                    nc.vector.tensor_mul(q[:, sl], pslice(pa, j), src)
                return q

            def finish(l3):
                acc = inter.tile([128, NJ * B], f32, tag="acc", name="acc")
                out_v = out.ap().rearrange("(j p) b -> j p b", p=128)
                for j in range(NJ):
                    sl = slice(j * B, (j + 1) * B)
                    nc.vector.tensor_copy(acc[:, sl], pslice(l3, j))
                    nc.scalar.dma_start(out_v[j], acc[:, sl])

            if role == "vis":
                load_act("mT")
                pc2 = stream_mm(act_view("mT"), "C2", "A")
                c2sb = evac_sbuf(pc2, "c2sb")
                load_act("hT")
                pu2 = stream_mm(act_view("mT"), "U2", "B")
                u2sb = evac_sbuf(pu2, "u2sb")
                load_act("v1T")
                load_act("v2T")
                pa = stream_mm(act_view("v1T"), "V1", "A")
                pb = stream_mm(act_view("v2T"), "V2", "B")
                q1 = had_mul(pa, bounce_from=pb, qname="q1")  # frees A and B
                pu = stream_mm(act_view("hT"), "U1", "A")     # fills H1 stall
                pa2 = stream_mm(q_view(q1), "C1", "B")
                q2 = had_mul(pa2, partner_sb=c2sb, qname="q2")  # frees B
                l3 = stream_mm(q_view(q2), "C3", "B", start_group=True,
                               stop_group=False)
                q3 = had_mul(pu, partner_sb=u2sb, qname="q3")  # frees A; || C3
                stream_mm(q_view(q3), "U3", None, psums=l3,
                          start_group=False, stop_group=True)
                finish(l3)
            else:
                load_act("mT")
                pw2 = stream_mm(act_view("mT"), "W2", "A")
                w2sb = evac_sbuf(pw2, "w2sb")
                load_act("xT")
                pa = stream_mm(act_view("xT"), "W1", "B")
                q = had_mul(pa, partner_sb=w2sb, qname="q1")   # frees B
                l3 = stream_mm(q_view(q), "W3", "A")
                finish(l3)

    nc.compile()
    return nc


def _make_runner(nc, devices):
    """Adapted from concourse.bass2jax.run_bass_via_pjrt: same lowering,
    but runs on an explicit device subset and returns unmaterialized jax
    arrays so two programs can be dispatched concurrently."""
    bass2jax.install_neuronx_cc_hook()

    assert nc.dbg_addr is None
    partition_name = (
        nc.partition_id_tensor.name if nc.partition_id_tensor else None
    )

    in_names, out_names, out_avals, zero_outs = [], [], [], []
    for alloc in nc.m.functions[0].allocations:
        if not isinstance(alloc, mybir.MemoryLocationSet):
            continue
        name = alloc.memorylocations[0].name
        if alloc.kind == "ExternalInput":
            if name != partition_name:
                in_names.append(name)
        elif alloc.kind == "ExternalOutput":
            shape = tuple(alloc.tensor_shape)
            dtype = mybir.dt.np(alloc.dtype)
            out_names.append(name)
            out_avals.append(jax.core.ShapedArray(shape, dtype))
            zero_outs.append(np.zeros(shape, dtype))
    n_params = len(in_names)
    n_outs = len(out_avals)
    in_names.extend(out_names)
    if partition_name is not None:
        in_names.append(partition_name)
    donate = tuple(range(n_params, n_params + n_outs))

    def _body(*args):
        operands = list(args)
        if partition_name is not None:
            operands.append(bass2jax.partition_id_tensor())
        outs = bass2jax._bass_exec_p.bind(
            *operands,
            out_avals=tuple(out_avals),
            in_names=tuple(in_names),
            out_names=tuple(out_names),
            lowering_input_output_aliases=(),
            sim_require_finite=True,
            sim_require_nnan=True,
            nc=nc,
        )
        return tuple(outs)

    n_cores = len(devices)
    mesh = Mesh(np.asarray(devices), ("core",))
    in_specs = (PartitionSpec("core"),) * (n_params + n_outs)
    out_specs = (PartitionSpec("core"),) * n_outs
    sharded = jax.jit(
        shard_map(
            _body, mesh=mesh, in_specs=in_specs, out_specs=out_specs,
            check_rep=False,
        ),
        donate_argnums=donate,
        keep_unused=True,
    )

    def run(in_maps):
        assert len(in_maps) == n_cores
        concat_in = [
            np.concatenate(
                [np.asarray(in_maps[c][name]) for c in range(n_cores)], axis=0
            )
            for name in in_names[:n_params]
        ]
        concat_zeros = [
            np.zeros((n_cores * z.shape[0], *z.shape[1:]), z.dtype)
            for z in zero_outs
        ]
        out_arrs = sharded(*concat_in, *concat_zeros)
        return out_names, out_avals, out_arrs

    return run


def _tile_actT(a, kdim):
    """[256 batch, K<=kdim] -> SBUF image [128, (kdim/128) * 256]:
    (p, t*256+b) = a[b, t*128+p], contiguous per partition."""
    ktiles = kdim // 128
    a = np.asarray(a, np.float32)
    if a.shape[1] < kdim:
        a = np.pad(a, ((0, 0), (0, kdim - a.shape[1])))
    # [256b, ktiles, 128p] -> [128p, ktiles, 256b]
    r = a.reshape(B, ktiles, 128).transpose(2, 1, 0)
    return np.ascontiguousarray(r.reshape(128, ktiles * B), dtype=_np_dt())


def kernel(prev_h, prev_c, x, m, v1, v2, V1, V2, C1, C2, C3, W1, W2, W3, U1, U2, U3, b):
    npdt = _np_dt()
    if "runners" not in _cache:
        devs = jax.devices()
        nc_vis = build_program("vis")
        nc_inp = build_program("inp")
        _cache["runners"] = (
            _make_runner(nc_vis, devs[0:4]),
            _make_runner(nc_inp, devs[4:8]),
        )
        _cache["ncs"] = (nc_vis, nc_inp)
    run_vis, run_inp = _cache["runners"]

    v1T_img = _tile_actT(v1, V)
    v2T_img = _tile_actT(v2, V)
    mT_img = _tile_actT(m, MM)
    hT_img = _tile_actT(prev_h, H2)
    xT_img = _tile_actT(x, XP)

    vis_maps, inp_maps = [], []
    for g in range(G):
        vis_maps.append({
            "v1T": v1T_img, "v2T": v2T_img, "mT": mT_img, "hT": hT_img,
            "V1": np.ascontiguousarray(V1[g], dtype=npdt),
            "V2": np.ascontiguousarray(V2[g], dtype=npdt),
            "C1": np.ascontiguousarray(C1[g], dtype=npdt),
            "C2": np.ascontiguousarray(C2[g], dtype=npdt),
            "C3": np.ascontiguousarray(C3[g], dtype=npdt),
            "U1": np.ascontiguousarray(U1[g], dtype=npdt),
            "U2": np.ascontiguousarray(U2[g], dtype=npdt),
            "U3": np.ascontiguousarray(U3[g], dtype=npdt),
        })
        w1_pad = np.zeros((XP, H1), np.float32)
        w1_pad[:X] = np.asarray(W1[g], np.float32)
        inp_maps.append({
            "xT": xT_img, "mT": mT_img,
            "W1": np.ascontiguousarray(w1_pad, dtype=npdt),
            "W2": np.ascontiguousarray(W2[g], dtype=npdt),
            "W3": np.ascontiguousarray(W3[g], dtype=npdt),
        })

    _cache["last_in_maps"] = (vis_maps, inp_maps)

    # dispatch both programs; they run concurrently on disjoint cores
    vnames, vavals, vouts = run_vis(vis_maps)
    inames, iavals, iouts = run_inp(inp_maps)

    # outputs are logits^T [G, H2, B]
    vis_out = np.asarray(vouts[0]).reshape(G, H1, B)
    inp_out = np.asarray(iouts[0]).reshape(G, H1, B)

    logits = (vis_out + inp_out).transpose(0, 2, 1) + \
        np.asarray(b, np.float32)[:, None, :]

    def sigmoid(z):
        return 1.0 / (1.0 + np.exp(-z))

    i = sigmoid(logits[0])
    f = sigmoid(logits[1])
    o = sigmoid(logits[2])
    cg = np.tanh(logits[3])
    prev_c = np.asarray(prev_c, np.float32)
    new_c = f * prev_c + i * cg
    new_h = o * np.tanh(prev_c)
    return new_h.astype(np.float32), new_c.astype(np.float32)


# revision 18
# speedup vs baseline: 1.8129x; 1.0409x over previous
"""DenseCaptioner LSTM-gate kernel for 8 Trainium2 NeuronCores.

Role-split sharding (no weight replication: each weight matrix is read
from HBM exactly once across the machine):
  cores 0-3  run program VIS: visual + recurrent paths for gate g = core,
             full batch  -> partial logits^T [1024,256]
  cores 4-7  run program INP: input path for gate g = core-4, full batch
             -> partial logits^T [1024,256]
Host: logits[g] = vis_part[g] + inp_part[g] + b[g], then sigmoid/tanh gate
math and the prev_c recurrence.

All matmul operands are bf16 (PSUM accumulation stays fp32): fp32r and
bf16 both stream 1 row/cycle on the TRN2 PE, so bf16's win is purely the
halved HBM traffic (emulated end-to-end rel err 4.6e-3 vs the 2e-2 gate).

Weight-stationary layout: every matmul uses a 128-column slice of the
streamed weight k-tile as the stationary lhsT and the [128,256]
activation k-tile image as the moving rhs, producing outputs directly in
[h-part, batch] layout. Hadamard products stay in that layout and feed
the next level as the moving rhs - no PE transposes, no identity, and
chunk-granular (128-row) pipelining across the hadamard boundaries. The
LDWEIGHTS pipe (8 x 128-row loads per k-tile) double-buffers under the
8 x 256-row matmuls.

Schedule (per core): independent m-projections (C2/U2 resp. W2) run
first into PSUM and are evacuated to SBUF - they keep the PE busy while
the big activation images stream in (activation DMAs are chunked per
4 k-tiles and issued from the otherwise-idle Activation queue so the
sync queue's ~600ns/DMA issue serialization doesn't gate startup), and
the later hadamards multiply PSUM x SBUF directly with no bounce copy.
The U1 stream is emitted between H1's DVE muls and dependent work to
fill that stall. C3+U3 share one open PSUM accumulation group. PSUM
budget: 2 tags x 4 slots x 1 bank (two 256-wide h-chunks per bank tile)
= all 8 banks.

The two programs are dispatched concurrently on disjoint device subsets
through a copy of concourse's PJRT runner that takes an explicit device
list (the stock one hardcodes jax.devices()[:n]).
"""

import numpy as np

import jax
from jax.experimental.shard_map import shard_map
from jax.sharding import Mesh, PartitionSpec

import concourse.mybir as mybir
import concourse.tile as tile
from concourse import bacc, bass2jax

B, X, V, MM, VH, H1, H2, G = 256, 12000, 4096, 1024, 1024, 1024, 1024, 4
XP = 12032  # X padded to a multiple of 128 (94 k-tiles)
N_CORES = 8
NJ = H1 // 128  # output h-chunks per stream

DT_NAME = "bfloat16"  # matmul dtype: "float32r" or "bfloat16"

_cache = {}


def _mm_dt():
    return getattr(mybir.dt, DT_NAME)


def _np_dt():
    return mybir.dt.np(_mm_dt())


def build_program(role):
    """role "vis": visual+recurrent paths; "inp": input path. Full batch."""
    dt = _mm_dt()
    f32 = mybir.dt.float32

    nc = bacc.Bacc("TRN2", target_bir_lowering=False, debug=False)

    if role == "vis":
        act_specs = {"mT": MM, "hT": H2, "v1T": V, "v2T": V}
        w_specs = {"V1": V, "V2": V, "C1": VH, "C2": MM, "C3": H1,
                   "U1": H2, "U2": MM, "U3": H1}
    else:
        act_specs = {"mT": MM, "xT": XP}
        w_specs = {"W1": XP, "W2": MM, "W3": H1}

    acts_d = {
        name: nc.dram_tensor(name, [128, k // 128 * B], dt, kind="ExternalInput")
        for name, k in act_specs.items()
    }
    # weights arrive host-interleaved as k-tile pairs: [K/2, 2*H1], so a
    # [128, 2*H1] tile DMA moves 4KB contiguous per partition
    wt = {
        name: nc.dram_tensor(name, [k // 2, 2 * H1], dt, kind="ExternalInput")
        for name, k in w_specs.items()
    }
    out = nc.dram_tensor("out", [H1, B], f32, kind="ExternalOutput")

    with tile.TileContext(nc) as tc:
        with (
            tc.tile_pool(name="acts", bufs=1) as acts,
            tc.tile_pool(name="wstream", bufs=8) as wstream,
            tc.tile_pool(name="inter", bufs=1) as inter,
            tc.tile_pool(name="ps", bufs=4, space="PSUM") as ps,
        ):
            act_sb = {}

            def load_act(name, chunk_kt=4):
                """Chunked resident activation load, [128, ktile, batch]
                image; issued on the Activation queue."""
                dram = acts_d[name]
                ktiles = act_specs[name] // 128
                t = acts.tile([128, ktiles * B], dt, tag=name, name=name)
                for c0 in range(0, ktiles, chunk_kt):
                    c1 = min(c0 + chunk_kt, ktiles)
                    nc.scalar.dma_start(
                        t[:, c0 * B:c1 * B], dram.ap()[:, c0 * B:c1 * B]
                    )
                act_sb[name] = t.rearrange("p (t x) -> p t x", x=B)

            def act_view(name):
                return lambda k: act_sb[name][:, k, :]

            def q_view(q):
                return lambda k: q[:, k * B:(k + 1) * B]

            # psum: [128, 512] f32 bank tiles, two 256-wide h-chunks each
            def pslice(psums, j):
                return psums[j // 2][:, (j % 2) * B:(j % 2 + 1) * B]

            def stream_mm(rhs, wname, ptag, psums=None, start_group=True,
                          stop_group=True, j_outer=False):
                """pslice(psums, j) [128, 256] (+)= W_ktile[:, j*128:...]^T
                @ rhs(k), streaming W k-tiles: weight columns stationary,
                activation image moving. Weights arrive as k-tile PAIRS
                (host-interleaved so each DMA descriptor is a contiguous
                4KB). j_outer completes output chunks progressively (for
                the final stream, so evac/store overlap the matmul tail)
                with all pair-tiles staged first."""
                ktiles = w_specs[wname] // 128
                w_dram = wt[wname].ap().rearrange("(t p) n -> t p n", p=128)
                if psums is None:
                    psums = [
                        ps.tile([128, 2 * B], f32, tag=ptag,
                                name=f"ps_{wname}{i}")
                        for i in range(NJ // 2)
                    ]

                def mm(k, j, w):
                    nc.tensor.matmul(
                        pslice(psums, j),
                        w[:, (k % 2) * H1 + j * 128:(k % 2) * H1 + (j + 1) * 128],
                        rhs(k),
                        # start zeroes the whole 2KB PSUM bank, so only the
                        # first write of each bank pair may set it
                        start=start_group and (k == 0) and (j % 2 == 0),
                        stop=stop_group and (k == ktiles - 1),
                    )

                if j_outer:
                    ws = []
                    for k2 in range(ktiles // 2):
                        w = wstream.tile([128, 2 * H1], dt, tag="w",
                                         name=f"w_{wname}{k2}")
                        nc.sync.dma_start(w[:], w_dram[k2])
                        ws.append(w)
                    for j in range(NJ):
                        for k in range(ktiles):
                            mm(k, j, ws[k // 2])
                else:
                    for k2 in range(ktiles // 2):
                        w = wstream.tile([128, 2 * H1], dt, tag="w",
                                         name=f"w_{wname}{k2}")
                        nc.sync.dma_start(w[:], w_dram[k2])
                        for k in (2 * k2, 2 * k2 + 1):
                            for j in range(NJ):
                                mm(k, j, w)
                return psums

            def evac_sbuf(psums, name):
                """Copy psum accumulators to a resident SBUF f32 image,
                alternating Vector/Activation engines to halve latency."""
                s = inter.tile([128, NJ * B], f32, tag=name, name=name)
                for j in range(NJ):
                    dst = s[:, j * B:(j + 1) * B]
                    if j % 2 == 0:
                        nc.vector.tensor_copy(dst, pslice(psums, j))
                    else:
                        nc.scalar.copy(dst, pslice(psums, j))
                return s

            def had_mul(pa, partner_sb=None, bounce_from=None, qname="q"):
                """q [128, NJ*256] bf16 = pa * partner, chunk-granular.
                partner: resident SBUF f32 image, or psum bounced via SBUF."""
                q = inter.tile([128, NJ * B], dt, tag="q", name=qname, bufs=2)
                bnc = None
                if partner_sb is None:
                    bnc = inter.tile([128, NJ * B], f32, tag="bounce",
                                     name=f"bounce_{qname}", bufs=2)
                for j in range(NJ):
                    sl = slice(j * B, (j + 1) * B)
                    if partner_sb is None:
                        # bounce on the Activation engine so the Vector muls
                        # pipeline right behind it
                        nc.scalar.copy(bnc[:, sl], pslice(bounce_from, j))
                        src = bnc[:, sl]
                    else:
                        src = partner_sb[:, sl]
                    nc.vector.tensor_mul(q[:, sl], pslice(pa, j), src)
                return q

            def finish(l3):
                acc = inter.tile([128, NJ * B], f32, tag="acc", name="acc")
                out_v = out.ap().rearrange("(j p) b -> j p b", p=128)
                for j in range(NJ):
                    sl = slice(j * B, (j + 1) * B)
                    if j % 2 == 0:
                        nc.vector.tensor_copy(acc[:, sl], pslice(l3, j))
                    else:
                        nc.scalar.copy(acc[:, sl], pslice(l3, j))
                    # sync queue is idle by now; keeps stores off the
                    # Activation queue which is doing the copies
                    nc.sync.dma_start(out_v[j], acc[:, sl])

            if role == "vis":
                load_act("mT")
                pc2 = stream_mm(act_view("mT"), "C2", "A")
                c2sb = evac_sbuf(pc2, "c2sb")
                load_act("hT")
                pu2 = stream_mm(act_view("mT"), "U2", "B")
                u2sb = evac_sbuf(pu2, "u2sb")
                load_act("v1T")
                load_act("v2T")
                pa = stream_mm(act_view("v1T"), "V1", "A")
                pb = stream_mm(act_view("v2T"), "V2", "B")
                q1 = had_mul(pa, bounce_from=pb, qname="q1")  # frees A and B
                # U1 in B: only waits H1's bounce copies, not its muls
                pu = stream_mm(act_view("hT"), "U1", "B")     # fills H1 stall
                pa2 = stream_mm(q_view(q1), "C1", "A")
                q2 = had_mul(pa2, partner_sb=c2sb, qname="q2")  # frees A
                l3 = stream_mm(q_view(q2), "C3", "A", start_group=True,
                               stop_group=False)
                q3 = had_mul(pu, partner_sb=u2sb, qname="q3")  # frees B; || C3
                stream_mm(q_view(q3), "U3", None, psums=l3,
                          start_group=False, stop_group=True, j_outer=True)
                finish(l3)
            else:
                load_act("mT")
                pw2 = stream_mm(act_view("mT"), "W2", "A")
                w2sb = evac_sbuf(pw2, "w2sb")
                load_act("xT")
                pa = stream_mm(act_view("xT"), "W1", "B")
                q = had_mul(pa, partner_sb=w2sb, qname="q1")   # frees B
                # k-inner: W3 row k consumes q chunk k as the muls emit them
                l3 = stream_mm(q_view(q), "W3", "A")
                finish(l3)

    nc.compile()
    return nc


def _make_runner(nc, devices):
    """Adapted from concourse.bass2jax.run_bass_via_pjrt: same lowering,
    but runs on an explicit device subset and returns unmaterialized jax
    arrays so two programs can be dispatched concurrently."""
    bass2jax.install_neuronx_cc_hook()

    assert nc.dbg_addr is None
    partition_name = (
        nc.partition_id_tensor.name if nc.partition_id_tensor else None
    )

    in_names, out_names, out_avals, zero_outs = [], [], [], []
    for alloc in nc.m.functions[0].allocations:
        if not isinstance(alloc, mybir.MemoryLocationSet):
            continue
        name = alloc.memorylocations[0].name
        if alloc.kind == "ExternalInput":
            if name != partition_name:
                in_names.append(name)
        elif alloc.kind == "ExternalOutput":
            shape = tuple(alloc.tensor_shape)
            dtype = mybir.dt.np(alloc.dtype)
            out_names.append(name)
            out_avals.append(jax.core.ShapedArray(shape, dtype))
            zero_outs.append(np.zeros(shape, dtype))
    n_params = len(in_names)
    n_outs = len(out_avals)
    in_names.extend(out_names)
    if partition_name is not None:
        in_names.append(partition_name)
    donate = tuple(range(n_params, n_params + n_outs))

    def _body(*args):
        operands = list(args)
        if partition_name is not None:
            operands.append(bass2jax.partition_id_tensor())
        outs = bass2jax._bass_exec_p.bind(
            *operands,
            out_avals=tuple(out_avals),
            in_names=tuple(in_names),
            out_names=tuple(out_names),
            lowering_input_output_aliases=(),
            sim_require_finite=True,
            sim_require_nnan=True,
            nc=nc,
        )
        return tuple(outs)

    n_cores = len(devices)
    mesh = Mesh(np.asarray(devices), ("core",))
    in_specs = (PartitionSpec("core"),) * (n_params + n_outs)
    out_specs = (PartitionSpec("core"),) * n_outs
    sharded = jax.jit(
        shard_map(
            _body, mesh=mesh, in_specs=in_specs, out_specs=out_specs,
            check_rep=False,
        ),
        donate_argnums=donate,
        keep_unused=True,
    )

    def run(in_maps):
        assert len(in_maps) == n_cores
        concat_in = [
            np.concatenate(
                [np.asarray(in_maps[c][name]) for c in range(n_cores)], axis=0
            )
            for name in in_names[:n_params]
        ]
        concat_zeros = [
            np.zeros((n_cores * z.shape[0], *z.shape[1:]), z.dtype)
            for z in zero_outs
        ]
        out_arrs = sharded(*concat_in, *concat_zeros)
        return out_names, out_avals, out_arrs

    return run


def _wpair(w):
    """[K, H] -> [K/2, 2H]: interleave k-tile pairs so each partition's DMA
    line is 4KB contiguous (rows p and p+128 of a tile pair adjacent)."""
    wk = np.asarray(w, np.float32)
    K_, H = wk.shape
    kt = K_ // 128
    r = wk.reshape(kt // 2, 2, 128, H).transpose(0, 2, 1, 3)
    return np.ascontiguousarray(r.reshape(K_ // 2, 2 * H), dtype=_np_dt())


def _tile_actT(a, kdim):
    """[256 batch, K<=kdim] -> SBUF image [128, (kdim/128) * 256]:
    (p, t*256+b) = a[b, t*128+p], contiguous per partition."""
    ktiles = kdim // 128
    a = np.asarray(a, np.float32)
    if a.shape[1] < kdim:
        a = np.pad(a, ((0, 0), (0, kdim - a.shape[1])))
    # [256b, ktiles, 128p] -> [128p, ktiles, 256b]
    r = a.reshape(B, ktiles, 128).transpose(2, 1, 0)
    return np.ascontiguousarray(r.reshape(128, ktiles * B), dtype=_np_dt())


def kernel(prev_h, prev_c, x, m, v1, v2, V1, V2, C1, C2, C3, W1, W2, W3, U1, U2, U3, b):
    npdt = _np_dt()
    if "runners" not in _cache:
        devs = jax.devices()
        nc_vis = build_program("vis")
        nc_inp = build_program("inp")
        _cache["runners"] = (
            _make_runner(nc_vis, devs[0:4]),
            _make_runner(nc_inp, devs[4:8]),
        )
        _cache["ncs"] = (nc_vis, nc_inp)
    run_vis, run_inp = _cache["runners"]

    v1T_img = _tile_actT(v1, V)
    v2T_img = _tile_actT(v2, V)
    mT_img = _tile_actT(m, MM)
    hT_img = _tile_actT(prev_h, H2)
    xT_img = _tile_actT(x, XP)

    vis_maps, inp_maps = [], []
    for g in range(G):
        vis_maps.append({
            "v1T": v1T_img, "v2T": v2T_img, "mT": mT_img, "hT": hT_img,
            "V1": _wpair(V1[g]), "V2": _wpair(V2[g]), "C1": _wpair(C1[g]),
            "C2": _wpair(C2[g]), "C3": _wpair(C3[g]), "U1": _wpair(U1[g]),
            "U2": _wpair(U2[g]), "U3": _wpair(U3[g]),
        })
        w1_pad = np.zeros((XP, H1), np.float32)
        w1_pad[:X] = np.asarray(W1[g], np.float32)
        inp_maps.append({
            "xT": xT_img, "mT": mT_img,
            "W1": _wpair(w1_pad),
            "W2": _wpair(W2[g]), "W3": _wpair(W3[g]),
        })

    _cache["last_in_maps"] = (vis_maps, inp_maps)

    # dispatch both programs; they run concurrently on disjoint cores
    vnames, vavals, vouts = run_vis(vis_maps)
    inames, iavals, iouts = run_inp(inp_maps)

    # outputs are logits^T [G, H2, B]
    vis_out = np.asarray(vouts[0]).reshape(G, H1, B)
    inp_out = np.asarray(iouts[0]).reshape(G, H1, B)

    logits = (vis_out + inp_out).transpose(0, 2, 1) + \
        np.asarray(b, np.float32)[:, None, :]

    def sigmoid(z):
        return 1.0 / (1.0 + np.exp(-z))

    i = sigmoid(logits[0])
    f = sigmoid(logits[1])
    o = sigmoid(logits[2])
    cg = np.tanh(logits[3])
    prev_c = np.asarray(prev_c, np.float32)
    new_c = f * prev_c + i * cg
    new_h = o * np.tanh(prev_c)
    return new_h.astype(np.float32), new_c.astype(np.float32)
